# revision 79
# baseline (speedup 1.0000x reference)
"""DCN-FPN Trainium2 kernel (nn_DCNFPN).

Sharding: 8 cores = 4 images x 2 row-halves. Each core computes rows
[g0, g0+23] of every 40-row intermediate (g0 = 0 top / 16 bottom), with
shrinking-validity redundancy so no cross-core communication is needed:
the correct-row front shrinks by 1 row per DCN iteration and we carry 4
spare rows; host keeps rows 0..19 (top) / 20..39 (bottom) of the output.

Per DCN call (4 calls: levels 0,1,0,1):
  - offset conv (3x3, 256->48) as 36 bf16 matmuls accumulating in PSUM
  - small math on [64,480] tiles (p = yx*32 + rcb*16 + tap) computes
    bilinear corner row-pair indices + 4 slot weights (mask/validity
    folded in; x-OOB handled by slot-remap E0/E1/F0 logic)
  - per tap (16 chunks): dma_gather of top/bot 2-pixel row pairs
    (bf16, elem 1KB) from the pixel-major feature table in DRAM,
    weight broadcast DMA, 7 DVE ops to combine corners, 8 matmuls
    accumulating dc in PSUM
  - f += relu(dc) in fp32 master, bf16 shadow for matmuls
Final: residual conv + fh, store [256, 960] fp32.

Sample enumeration: i = tap*960 + rc, rc = rcb*480 + c (rcb in {0,1}).
Gather idx layout [i%16, i//16] == [rc%16, tap*60 + rcb*30 + c//16].
"""
import sys
sys.path.insert(0, "/opt/trn_rl_repo")

from contextlib import ExitStack
import numpy as np
import ml_dtypes

import os
import bass_rust
import concourse.bass as bass
import concourse.bacc as bacc
import concourse.mybir as mybir
import concourse.tile as tile

F32 = mybir.dt.float32
BF16 = mybir.dt.bfloat16
I16 = mybir.dt.int16
I32 = mybir.dt.int32
A = mybir.AluOpType
AF = mybir.ActivationFunctionType

B, C, HOUT = 4, 256, 40
CONFIGS = [(4, 2, 1, 1), (4, 4, 3, 3)]   # (k, stride, pad, dil)
HIN = [80, 160]                          # per level l=0 (f1), l=1 (f0)
ROWS = 24                                # out rows per core per call
RC = ROWS * HOUT                         # 960
NT = 16                                  # taps
CALLS = [0, 1, 0, 1]
FW = 42                                  # padded f width
FR = 26                                  # f window rows
FSZ = FR * FW                            # 1092


def ap_with(ap, dims, offset_elems=None):
    v = ap.copy()
    if offset_elems is not None:
        v = v  # offset handled by caller slicing
    v.ap = bass_rust.VecI64Pair(dims)
    return v


def build_program():
    nc = bacc.Bacc("TRN2", target_bir_lowering=False, debug=False)

    dt = {}

    def din(name, shape, dtype=F32):
        dt[name] = nc.dram_tensor(name, shape, dtype, kind="ExternalInput").ap()

    din("fp0", [HIN[1] * HIN[1] + 1, C], BF16)
    din("fp1", [HIN[0] * HIN[0] + 1, C], BF16)
    din("finit", [C, FSZ], F32)
    din("finitb", [C, FSZ], BF16)
    din("fh", [128, 2 * RC], BF16)
    din("byx", [64, 2 * 480], F32)
    din("hi0", [64, 2], F32)
    din("oneh", [32, 32 * 128], BF16)
    din("com_w", [128, 2 * 9 * 2 * 48], BF16)
    din("com_b", [48, 2], F32)
    din("dcn_w", [2, 128, NT * 2 * 2 * 128], BF16)
    din("dcn_b", [128, 4], F32)
    din("res_w", [128, 9 * 2 * 2 * 128], BF16)
    din("res_b", [128, 2], F32)
    out_d = nc.dram_tensor("out", [C, RC], F32, kind="ExternalOutput").ap()

    with tile.TileContext(nc) as tc, ExitStack() as ctx:
        build_body(nc, tc, ctx, dt, out_d)
    nc.compile()
    return nc


def build_body(nc, tc, ctx, dt, out_d):
    cst = ctx.enter_context(tc.tile_pool(name="cst", bufs=1))
    s64p = ctx.enter_context(tc.tile_pool(name="s64p", bufs=10))
    s32p = ctx.enter_context(tc.tile_pool(name="s32p", bufs=5))
    i32p = ctx.enter_context(tc.tile_pool(name="i32p", bufs=1))
    smi = ctx.enter_context(tc.tile_pool(name="smi", bufs=2))
    omp = ctx.enter_context(tc.tile_pool(name="omp", bufs=1))
    wgt = ctx.enter_context(tc.tile_pool(name="wgt", bufs=1))
    gat = ctx.enter_context(tc.tile_pool(name="gat", bufs=2))
    wbp = ctx.enter_context(tc.tile_pool(name="wbp", bufs=3))
    pp = ctx.enter_context(tc.tile_pool(name="pp", bufs=2))
    qp = ctx.enter_context(tc.tile_pool(name="qp", bufs=2))
    fup = ctx.enter_context(tc.tile_pool(name="fup", bufs=1))
    psd = ctx.enter_context(tc.tile_pool(name="psd", bufs=1, space="PSUM"))
    wbq = ctx.enter_context(tc.tile_pool(name="wbq", bufs=2, space="PSUM"))
    drp = ctx.enter_context(tc.tile_pool(name="drp", bufs=2, space="DRAM"))

    # ---- persistent loads (finit first: first om conv depends on it) ------
    fmas, fsh = [], []
    for h in range(2):
        fs = cst.tile([128, FSZ], BF16, tag=f"fsh{h}")
        nc.sync.dma_start(fs[:], dt["finitb"][128 * h:128 * (h + 1), :])
        fm = cst.tile([128, FSZ], F32, tag=f"fmas{h}")
        nc.sync.dma_start(fm[:], dt["finit"][128 * h:128 * (h + 1), :])
        fmas.append(fm)
        fsh.append(fs)

    com_t = cst.tile([128, 2 * 9 * 2 * 48], BF16, tag="com")
    nc.sync.dma_start(com_t[:], dt["com_w"])
    com_v = com_t[:].rearrange("p (l t i o) -> p l t i o", l=2, t=9, i=2, o=48)

    byx_t = cst.tile([64, 2 * 480], F32, tag="byx")
    nc.sync.dma_start(byx_t[:], dt["byx"])
    hi0_t = cst.tile([64, 2], F32, tag="hi0")
    nc.sync.dma_start(hi0_t[:], dt["hi0"])
    comb_t = cst.tile([48, 2], F32, tag="comb")
    nc.sync.dma_start(comb_t[:], dt["com_b"])
    dcnb_t = cst.tile([128, 4], F32, tag="dcnb")
    nc.sync.dma_start(dcnb_t[:], dt["dcn_b"])
    resb_t = cst.tile([128, 2], F32, tag="resb")
    nc.sync.dma_start(resb_t[:], dt["res_b"])
    fh_t = cst.tile([128, 2 * RC], BF16, tag="fh")
    nc.sync.dma_start(fh_t[:], dt["fh"])

    # persistent gather-idx tile, both corners: [128, (corner, t, z, cc)]
    rep2 = cst.tile([128, 2 * NT * 64], I16, tag="rep2")
    nc.vector.memset(rep2[:], 0)

    # one-hot row selectors for the PE weight broadcast:
    # oneh[p, zt*128 + o] = (p == zt)
    oneh = cst.tile([32, 32 * 128], BF16, tag="oneh")
    nc.sync.dma_start(oneh[:], dt["oneh"])

    # residual conv weights preloaded (keeps the final conv off the
    # critical path of the last call's weight-slot WAR)
    res_t = cst.tile([128, 9 * 2 * 2 * 128], BF16, tag="res")
    nc.sync.dma_start(res_t[:], dt["res_w"])



    fp_ap = {0: dt["fp1"], 1: dt["fp0"]}

    # ---- per-call ---------------------------------------------------------
    for ci, lvl in enumerate(CALLS):
        k_, st_, pad_, dil_ = CONFIGS[lvl]
        Hin = Win = HIN[lvl]

        # per-call DCN weights
        dcn_t = wgt.tile([128, NT * 2 * 2 * 128], BF16, tag="dcn")
        nc.sync.dma_start(dcn_t[:], dt["dcn_w"][lvl])
        dcn_v = dcn_t[:].rearrange("p (k i o q) -> p k i o q", k=NT, i=2, o=2, q=128)

        # offset conv: om_ps rows 0:48 of a wbq psum slot, [48, (z, 480-of-512)]
        om_full = wbq.tile([128, 1024], F32, tag="wbps", name=f"om_{ci}")
        om_ps = om_full[0:48, :]
        conv3x3(nc, fsh, lambda ti, ih: com_v[:, lvl, ti, ih], om_ps)

        om01 = omp.tile([32, RC], F32, tag="om01")
        omv0 = om_ps[0:32, :].rearrange("p (z c) -> p z c", z=2)[:, :, 0:480]
        nc.scalar.activation(om01[:], omv0, AF.Identity,
                             bias=comb_t[0:32, lvl:lvl + 1])
        m16 = omp.tile([16, RC], F32, tag="m16")
        omv1 = om_ps[32:48, :].rearrange("p (z c) -> p z c", z=2)[:, :, 0:480]
        nc.scalar.activation(m16[:], omv1, AF.Sigmoid,
                             bias=comb_t[32:48, lvl:lvl + 1])

        # shuffle into [64,480] (p = yx*32 + rcb*16 + t) / [32,480]
        pos0 = s64p.tile([64, 480], F32, tag="s64")
        for yx in range(2):
            for rcb in range(2):
                nc.sync.dma_start(
                    pos0[yx * 32 + rcb * 16: yx * 32 + rcb * 16 + 16, :],
                    om01[yx * 16:yx * 16 + 16, rcb * 480:(rcb + 1) * 480])
        m32 = s32p.tile([32, 480], F32, tag="s32")
        for rcb in range(2):
            nc.sync.dma_start(m32[rcb * 16:rcb * 16 + 16, :],
                              m16[:, rcb * 480:(rcb + 1) * 480])

        # ---- small math ----
        cnt = [0]

        def t64():
            cnt[0] += 1
            return s64p.tile([64, 480], F32, tag="s64", name=f"t64_{ci}_{cnt[0]}")

        def t32():
            cnt[0] += 1
            return s32p.tile([32, 480], F32, tag="s32", name=f"t32_{ci}_{cnt[0]}")

        pos = t64()
        nc.vector.tensor_tensor(pos[:], pos0[:],
                                byx_t[:, lvl * 480:(lvl + 1) * 480], A.add)
        sh = t64()
        nc.vector.tensor_scalar(sh[:], pos[:], 1024.0, None, A.add)
        i32t = i32p.tile([64, 480], I32, tag="i32")
        nc.vector.tensor_scalar(i32t[:], pos[:], 1024.0, None, A.add)
        ff = t64()
        nc.vector.tensor_copy(ff[:], i32t[:])
        gt = t64()
        nc.vector.tensor_tensor(gt[:], ff[:], sh[:], A.is_gt)
        fl = t64()
        nc.vector.scalar_tensor_tensor(fl[:], ff[:], -1024.0, gt[:], A.add, A.subtract)
        frac = t64()
        nc.vector.tensor_tensor(frac[:], pos[:], fl[:], A.subtract)
        c0 = t64()
        nc.vector.tensor_scalar(c0[:], fl[:], 0.0, hi0_t[:, lvl:lvl + 1],
                                A.max, A.min)
        t1 = t64()
        nc.vector.tensor_scalar(t1[:], fl[:], 1.0, None, A.add)
        cc1 = t64()
        nc.vector.tensor_scalar(cc1[0:32, :], t1[0:32, :], 0.0, float(Hin - 1),
                                A.max, A.min)
        nc.vector.tensor_copy(cc1[32:64, :], c0[32:64, :])

        # ---- gather indices (emitted early: DVE runs in-order, so the idx
        # DMA chain overlaps the rest of the small math) ----
        # gy written in c' = p'*30+cc column order so DRAM runs are contiguous
        def cin(ap):    # [32,480] rc-ordered input viewed in (p', cc) order
            return ap.rearrange("p (cc pp) -> p pp cc", cc=30, pp=16)

        psx = t32()
        nc.vector.tensor_copy(psx[:], c0[32:64, :])
        gy2 = omp.tile([32, 960], F32, tag="gy2")
        for corner, cy in ((0, c0), (1, cc1)):
            gyv = gy2[:, corner * 480:(corner + 1) * 480].rearrange(
                "p (pp cc) -> p pp cc", pp=16, cc=30)
            nc.vector.scalar_tensor_tensor(gyv, cin(cy[0:32, :]), float(Win),
                                           cin(psx[:]), A.mult, A.add)
        i16t = smi.tile([32, 960], I16, tag="i16")
        nc.scalar.copy(i16t[:], gy2[:])
        # DRAM blocks (corner*2+z) of [p' 16][t 16][cc 30]; dump + wrap with
        # 8x in-DMA replication to all 128 idx partitions (no doubling chain)
        dflat = drp.tile([32, 960], I16, tag="dfl")
        dfv = dflat[:].rearrange("p c -> (p c)")
        r2v = rep2[:].rearrange("p (k t z cc) -> p k t z cc",
                                k=2, t=NT, z=2, cc=32)
        for corner in range(2):
            for z in range(2):
                boff = (corner * 2 + z) * 7680
                dst = dfv[boff:boff + 1].copy()
                dst.ap = bass_rust.VecI64Pair([[30, 16], [480, 16], [1, 30]])
                nc.sync.dma_start(
                    dst, i16t[z * 16:(z + 1) * 16,
                              corner * 480:(corner + 1) * 480])
                src = dfv[boff:boff + 1].copy()
                src.ap = bass_rust.VecI64Pair([[0, 8], [480, 16], [1, 480]])
                nc.sync.dma_start(r2v[:, corner, :, z, 0:30], src)

        V0 = t64()
        nc.vector.tensor_tensor(V0[:], c0[:], fl[:], A.is_equal)
        V1 = t64()
        nc.vector.tensor_tensor(V1[:], cc1[:], t1[:], A.is_equal)
        # x-side ops stay on base-partition 32 (walrus: TT inputs must share
        # base partition); results copied down to base 0 where y-side needs them
        psp = t64()
        nc.vector.tensor_scalar(psp[32:64, :], c0[32:64, :], 1.0, None, A.add)
        F0 = t64()
        nc.vector.tensor_tensor(F0[32:64, :], psp[32:64, :], fl[32:64, :], A.is_equal)
        u = t64()
        nc.vector.tensor_scalar(u[:], frac[:], -1.0, 1.0, A.mult, A.add)
        P0 = t64()
        nc.vector.tensor_tensor(P0[:], u[:], V0[:], A.mult)
        P1 = t64()
        nc.vector.tensor_tensor(P1[:], frac[:], V1[:], A.mult)
        xsA = t64()
        nc.vector.tensor_tensor(xsA[32:64, :], P0[32:64, :], P1[32:64, :], A.add)
        xt = t64()
        nc.vector.tensor_tensor(xt[32:64, :], F0[32:64, :], u[32:64, :], A.mult)
        xt2 = t64()
        nc.vector.tensor_tensor(xt2[32:64, :], V0[32:64, :], frac[32:64, :], A.mult)
        xsB = t64()
        nc.vector.tensor_tensor(xsB[32:64, :], xt[32:64, :], xt2[32:64, :], A.add)
        xs0 = t32()
        nc.vector.tensor_copy(xs0[:], xsA[32:64, :])
        xs1 = t32()
        nc.vector.tensor_copy(xs1[:], xsB[32:64, :])
        A0 = t32()
        nc.vector.tensor_tensor(A0[:], P0[0:32, :], m32[:], A.mult)
        A1 = t32()
        nc.vector.tensor_tensor(A1[:], P1[0:32, :], m32[:], A.mult)

        # wall [32, (slot j, c512)] bf16, j = A(top,pix0) B(top,pix1) C D (bot)
        # 512-col blocks: 480 real + 32 zero pad (sample space padded to 1024/tap)
        wall = smi.tile([32, 4 * 512], BF16, tag="wall")
        nc.vector.tensor_tensor(wall[:, 0 * 512:0 * 512 + 480], A0[:], xs0[:], A.mult)
        nc.vector.tensor_tensor(wall[:, 1 * 512:1 * 512 + 480], A0[:], xs1[:], A.mult)
        nc.vector.tensor_tensor(wall[:, 2 * 512:2 * 512 + 480], A1[:], xs0[:], A.mult)
        nc.vector.tensor_tensor(wall[:, 3 * 512:3 * 512 + 480], A1[:], xs1[:], A.mult)

        # dc accumulator [2][128, 960]
        dcs = [psd.tile([128, 1024], F32, tag=f"dc{oh}", name=f"dc_{ci}_{oh}")
               for oh in range(2)]

        fpv = fp_ap[lvl].copy()
        fpv.ap = bass_rust.VecI64Pair([[C, Hin * Win], [1, 2 * C]])

        RCP = 1024
        for t in range(NT):
            # weight broadcast via PE one-hot matmul + Act drain:
            # wb free (j, z, c512) = wall[z*16+t, j*512+c] on all 128 parts
            wb = wbp.tile([128, 4 * RCP], BF16, tag="wb")
            wb4 = wb[:].rearrange("p (j z c) -> p j z c", j=4, z=2, c=512)
            for z in range(2):
                zt = z * 16 + t
                for jp in range(2):
                    wbps = wbq.tile([128, 1024], F32, tag="wbps",
                                    name=f"wbps_{ci}_{t}_{z}_{jp}")
                    for jh in range(2):
                        nc.tensor.matmul(
                            wbps[:, jh * 512:jh * 512 + 480],
                            oneh[:, zt * 128:(zt + 1) * 128],
                            wall[:, jp * 1024 + jh * 512:
                                 jp * 1024 + jh * 512 + 480],
                            start=True, stop=True)
                    wpv = wbps[:].rearrange("p (j c) -> p j c", j=2)
                    nc.scalar.copy(wb4[:, 2 * jp:2 * jp + 2, z, 0:480],
                                   wpv[:, :, 0:480])
            wbv = wb[:].rearrange("p (j zc) -> p j zc", j=4)

            g2 = gat.tile([128, 2 * 4 * RCP], BF16, tag="g2")
            for corner in range(2):
                gv = g2[:, corner * 4096:(corner + 1) * 4096].rearrange(
                    "p (j i) -> p j i", j=4)
                ioff = corner * NT * 64 + t * 64
                nc.gpsimd.dma_gather(gv, fpv, rep2[:, ioff:ioff + 64],
                                     RCP, RCP, 2 * C, elem_step=C,
                                     transpose=True, single_packet=False)
            g4 = g2[:].rearrange("p (k pix hl z c) -> p k pix hl z c",
                                 k=2, pix=2, hl=2, z=2)
            wbk = wb[:].rearrange("p (k pix z c) -> p k pix z c",
                                  k=2, pix=2, z=2)

            # pad-skip: all DVE ops process only the 480 real cols per z-block
            ss = []
            for hilo in range(2):
                p2 = pp.tile([128, 4 * RCP], BF16, tag="p")
                pv = p2[:].rearrange("p (k pix z c) -> p k pix z c",
                                     k=2, pix=2, z=2)
                nc.vector.tensor_tensor(pv[:, :, :, :, 0:480],
                                        g4[:, :, :, hilo, :, 0:480],
                                        wbk[:, :, :, :, 0:480], A.mult)
                q = qp.tile([128, 2 * RCP], BF16, tag="q")
                qv = q[:].rearrange("p (j z c) -> p j z c", j=2, z=2)
                nc.vector.tensor_tensor(qv[:, :, :, 0:480],
                                        pv[:, 0, :, :, 0:480],
                                        pv[:, 1, :, :, 0:480], A.add)
                ss.append(q)

            # pix-sum absorbed into PE: two matmuls (pix slices of q)
            # accumulate into the same PSUM region
            for oh in range(2):
                for ih in range(2):
                    for nh in range(2):
                        for pix in range(2):
                            nc.tensor.matmul(
                                dcs[oh][:, nh * 512:nh * 512 + 480],
                                dcn_v[:, t, ih, oh],
                                ss[ih][:, pix * 1024 + nh * 512:
                                       pix * 1024 + nh * 512 + 480],
                                start=(t == 0 and ih == 0 and pix == 0),
                                stop=(t == NT - 1 and ih == 1 and pix == 1))

        # f update: f += relu(dc + b); z-major halves so the next call's om
        # conv (nh0 matmuls) can start as soon as rows 1:13 are final
        rels = []
        for h in range(2):
            rel = fup.tile([128, RC], F32, tag=f"rel{h}")
            dcv = dcs[h][:].rearrange("p (z c) -> p z c", z=2)[:, :, 0:480]
            nc.scalar.activation(rel[:], dcv, AF.Relu,
                                 bias=dcnb_t[:, 2 * lvl + h:2 * lvl + h + 1])
            rels.append(rel)
        for zh in range(2):
            for h in range(2):
                fiv = fmas[h][:].rearrange("p (r c) -> p r c", c=FW)[
                    :, 1 + 12 * zh:13 + 12 * zh, 1:41]
                rv = rels[h][:].rearrange("p (r c) -> p r c", c=HOUT)[
                    :, 12 * zh:12 * zh + 12, :]
                nc.vector.tensor_tensor(fiv, fiv, rv, A.add)
                fsv = fsh[h][:].rearrange("p (r c) -> p r c", c=FW)[
                    :, 1 + 12 * zh:13 + 12 * zh, 1:41]
                nc.scalar.copy(fsv, fiv)

    # ---- residual conv + fh ----------------------------------------------
    res_v = res_t[:].rearrange("p (t i o q) -> p t i o q", t=9, i=2, o=2)
    for oh in range(2):
        rps = psd.tile([128, 1024], F32, tag=f"dc{oh}")
        conv3x3(nc, fsh, lambda ti, ih, oh=oh: res_v[:, ti, ih, oh], rps)
        ot = fup.tile([128, RC], F32, tag="ot")
        rpv = rps[:].rearrange("p (z c) -> p z c", z=2)[:, :, 0:480]
        nc.scalar.activation(ot[:], rpv, AF.Identity, bias=resb_t[:, oh:oh + 1])
        nc.vector.tensor_tensor(ot[:], ot[:], fh_t[:, oh * RC:(oh + 1) * RC], A.add)
        nc.sync.dma_start(out_d[128 * oh:128 * (oh + 1), :], ot[:])


def conv3x3(nc, fsh, w_fn, out_ps):
    """3x3 stride-1 conv over the padded f window; out [cout, (z,480-of-512)]
    (PSUM bank limit: a matmul output cannot cross a 512-f32 bank).
    Emitted nh-major, dy-sorted, so the nh0/dy<=0 matmuls only depend on the
    first half of the f update."""
    taps = [(a, b) for a in (-1, 0, 1) for b in (-1, 0, 1)]
    seq = [(ti, dy, dx, ih, nh)
           for nh in range(2)
           for ti, (dy, dx) in enumerate(taps)
           for ih in range(2)]
    for k, (ti, dy, dx, ih, nh) in enumerate(seq):
        rhs = fsh[ih][:].rearrange("p (r c) -> p r c", c=FW)
        nc.tensor.matmul(
            out_ps[:, nh * 512:nh * 512 + 480],
            w_fn(ti, ih),
            rhs[:, 1 + dy + nh * 12:1 + dy + nh * 12 + 12,
                1 + dx:1 + dx + 40],
            start=(ti == 0 and ih == 0), stop=(ti == 8 and ih == 1))


# ===========================================================================
# host side
# ===========================================================================

def prep_core_inputs(inputs, b, half):
    """Per-core input map for image b, row-half `half` (0=top)."""
    g0 = 0 if half == 0 else 16
    f0 = np.asarray(inputs["f0"][b], np.float32)
    f1 = np.asarray(inputs["f1"][b], np.float32)
    f2 = np.asarray(inputs["f2"][b], np.float32)

    def pix_table(f):
        hw = f.shape[1] * f.shape[2]
        t = np.zeros((hw + 1, C), np.float32)
        t[:hw] = f.transpose(1, 2, 0).reshape(hw, C)
        return t.astype(ml_dtypes.bfloat16)

    finit = np.zeros((C, FR, FW), np.float32)
    for r in range(FR):
        gr = g0 - 1 + r
        if 0 <= gr < HOUT:
            finit[:, r, 1:41] = f2[:, gr, :]

    # fh as [128, (oh, rc)]
    fh0 = f2[:, g0:g0 + ROWS, :].reshape(C, RC)
    fh = np.concatenate([fh0[:128], fh0[128:]], axis=1)

    byx = np.zeros((2, 64, 480), np.float32)
    hi0 = np.zeros((2, 64, 1), np.float32)
    for lvl in range(2):
        k_, st_, pad_, dil_ = CONFIGS[lvl]
        Hin = HIN[lvl]
        rc = np.arange(480)
        for rcb in range(2):
            rr = (rcb * 480 + rc) // HOUT
            cc = (rcb * 480 + rc) % HOUT
            for t in range(NT):
                byx[lvl, rcb * 16 + t] = st_ * (g0 + rr) - pad_ + (t // k_) * dil_
                byx[lvl, 32 + rcb * 16 + t] = st_ * cc - pad_ + (t % k_) * dil_
        hi0[lvl, 0:32] = Hin - 1
        hi0[lvl, 32:64] = Hin - 2
    byx = byx.transpose(1, 0, 2).reshape(64, 2 * 480)
    hi0 = hi0.transpose(1, 0, 2).reshape(64, 2)

    perm = list(range(0, 32, 2)) + list(range(1, 32, 2)) + list(range(32, 48))
    com_w = np.zeros((2, 9, 2, 128, 48), np.float32)
    com_b = np.zeros((2, 48, 1), np.float32)
    dcn_w = np.zeros((2, NT, 2, 2, 128, 128), np.float32)
    dcn_b = np.zeros((2, 2, 128, 1), np.float32)
    for lvl in range(2):
        cw = np.asarray(inputs[f"com_w{lvl}"], np.float32)[perm]
        cb = np.asarray(inputs[f"com_b{lvl}"], np.float32)[perm]
        for ty in range(3):
            for tx in range(3):
                for ih in range(2):
                    com_w[lvl, ty * 3 + tx, ih] = \
                        cw[:, ih * 128:(ih + 1) * 128, ty, tx].T
        com_b[lvl, :, 0] = cb
        dw = np.asarray(inputs[f"dcn_w{lvl}"], np.float32)
        for k in range(NT):
            for ih in range(2):
                for oh in range(2):
                    dcn_w[lvl, k, ih, oh] = dw[oh * 128:(oh + 1) * 128,
                                               ih * 128:(ih + 1) * 128,
                                               k // 4, k % 4].T
        db = np.asarray(inputs[f"dcn_b{lvl}"], np.float32)
        dcn_b[lvl, 0, :, 0] = db[:128]
        dcn_b[lvl, 1, :, 0] = db[128:]
    rw = np.asarray(inputs["res_w"], np.float32)
    res_w = np.zeros((9, 2, 2, 128, 128), np.float32)
    for ty in range(3):
        for tx in range(3):
            for ih in range(2):
                for oh in range(2):
                    res_w[ty * 3 + tx, ih, oh] = rw[oh * 128:(oh + 1) * 128,
                                                    ih * 128:(ih + 1) * 128,
                                                    ty, tx].T
    rb = np.asarray(inputs["res_b"], np.float32)
    res_b = np.stack([rb[:128], rb[128:]], axis=1)  # [128, 2]

    # transpose weight stacks to [partition, ...] DRAM layouts
    com_w = com_w.transpose(3, 0, 1, 2, 4).reshape(128, -1)
    com_b = com_b.transpose(1, 0, 2).reshape(48, 2)
    dcn_w = dcn_w.transpose(0, 4, 1, 2, 3, 5).reshape(2, 128, -1)
    dcn_b = dcn_b.transpose(2, 0, 1, 3).reshape(128, 4)
    res_w = res_w.transpose(3, 0, 1, 2, 4).reshape(128, -1)

    oneh = np.zeros((32, 32 * 128), np.float32)
    for zt in range(32):
        oneh[zt, zt * 128:(zt + 1) * 128] = 1.0

    return {
        "oneh": oneh.astype(ml_dtypes.bfloat16),
        "fp0": pix_table(f0),
        "fp1": pix_table(f1),
        "finit": finit.reshape(C, FSZ),
        "finitb": finit.reshape(C, FSZ).astype(ml_dtypes.bfloat16),
        "fh": fh.astype(ml_dtypes.bfloat16),
        "byx": byx,
        "hi0": hi0,
        "com_w": com_w.astype(ml_dtypes.bfloat16),
        "com_b": np.ascontiguousarray(com_b),
        "dcn_w": np.ascontiguousarray(dcn_w).astype(ml_dtypes.bfloat16),
        "dcn_b": np.ascontiguousarray(dcn_b),
        "res_w": np.ascontiguousarray(res_w).astype(ml_dtypes.bfloat16),
        "res_b": np.ascontiguousarray(res_b).astype(np.float32),
    }


def assemble_output(results):
    out = np.zeros((B, C, HOUT, HOUT), np.float32)
    for b in range(B):
        top = np.asarray(results[2 * b]["out"]).reshape(C, ROWS, HOUT)
        bot = np.asarray(results[2 * b + 1]["out"]).reshape(C, ROWS, HOUT)
        out[b, :, 0:20, :] = top[:, 0:20, :]
        out[b, :, 20:40, :] = bot[:, 4:24, :]
    return out


_NC_CACHE = []


def kernel(**inputs):
    if not _NC_CACHE:
        _NC_CACHE.append(build_program())
    nc = _NC_CACHE[0]
    in_maps = [prep_core_inputs(inputs, b, half)
               for b in range(B) for half in range(2)]
    from concourse.bass_utils import run_bass_kernel_spmd
    r = run_bass_kernel_spmd(nc, in_maps, list(range(8)))
    return assemble_output(r.results)



# revision 80
# speedup vs baseline: 1.0082x; 1.0082x over previous
"""DCN-FPN Trainium2 kernel (nn_DCNFPN).

Sharding: 8 cores = 4 images x 2 row-halves. Each core computes rows
[g0, g0+23] of every 40-row intermediate (g0 = 0 top / 16 bottom), with
shrinking-validity redundancy so no cross-core communication is needed:
the correct-row front shrinks by 1 row per DCN iteration and we carry 4
spare rows; host keeps rows 0..19 (top) / 20..39 (bottom) of the output.

Per DCN call (4 calls: levels 0,1,0,1):
  - offset conv (3x3, 256->48) as 36 bf16 matmuls accumulating in PSUM
  - small math on [64,480] tiles (p = yx*32 + rcb*16 + tap) computes
    bilinear corner row-pair indices + 4 slot weights (mask/validity
    folded in; x-OOB handled by slot-remap E0/E1/F0 logic)
  - per tap (16 chunks): dma_gather of top/bot 2-pixel row pairs
    (bf16, elem 1KB) from the pixel-major feature table in DRAM,
    weight broadcast DMA, 7 DVE ops to combine corners, 8 matmuls
    accumulating dc in PSUM
  - f += relu(dc) in fp32 master, bf16 shadow for matmuls
Final: residual conv + fh, store [256, 960] fp32.

Sample enumeration: i = tap*960 + rc, rc = rcb*480 + c (rcb in {0,1}).
Gather idx layout [i%16, i//16] == [rc%16, tap*60 + rcb*30 + c//16].
"""
import sys
sys.path.insert(0, "/opt/trn_rl_repo")

from contextlib import ExitStack
import numpy as np
import ml_dtypes

import os
import bass_rust
import concourse.bass as bass
import concourse.bacc as bacc
import concourse.mybir as mybir
import concourse.tile as tile

F32 = mybir.dt.float32
BF16 = mybir.dt.bfloat16
I16 = mybir.dt.int16
I32 = mybir.dt.int32
A = mybir.AluOpType
AF = mybir.ActivationFunctionType

B, C, HOUT = 4, 256, 40
CONFIGS = [(4, 2, 1, 1), (4, 4, 3, 3)]   # (k, stride, pad, dil)
HIN = [80, 160]                          # per level l=0 (f1), l=1 (f0)
ROWS = 24                                # out rows per core per call
RC = ROWS * HOUT                         # 960
NT = 16                                  # taps
CALLS = [0, 1, 0, 1]
FW = 42                                  # padded f width
FR = 26                                  # f window rows
FSZ = FR * FW                            # 1092


def ap_with(ap, dims, offset_elems=None):
    v = ap.copy()
    if offset_elems is not None:
        v = v  # offset handled by caller slicing
    v.ap = bass_rust.VecI64Pair(dims)
    return v


def build_program():
    nc = bacc.Bacc("TRN2", target_bir_lowering=False, debug=False)

    dt = {}

    def din(name, shape, dtype=F32):
        dt[name] = nc.dram_tensor(name, shape, dtype, kind="ExternalInput").ap()

    din("fp0", [HIN[1] * HIN[1] + 1, C], BF16)
    din("fp1", [HIN[0] * HIN[0] + 1, C], BF16)
    din("finit", [C, FSZ], F32)
    din("fh", [128, 2 * RC], BF16)
    din("byx", [64, 2 * 480], F32)
    din("hi0", [64, 2], F32)
    din("oneh", [32, 32 * 128], BF16)
    din("com_w", [128, 2 * 9 * 2 * 48], BF16)
    din("com_b", [48, 2], F32)
    din("dcn_w", [2, 128, NT * 2 * 2 * 128], BF16)
    din("dcn_b", [128, 4], F32)
    din("res_w", [128, 9 * 2 * 2 * 128], BF16)
    din("res_b", [128, 2], F32)
    out_d = nc.dram_tensor("out", [C, RC], F32, kind="ExternalOutput").ap()

    with tile.TileContext(nc) as tc, ExitStack() as ctx:
        build_body(nc, tc, ctx, dt, out_d)
    nc.compile()
    return nc


def build_body(nc, tc, ctx, dt, out_d):
    cst = ctx.enter_context(tc.tile_pool(name="cst", bufs=1))
    s64p = ctx.enter_context(tc.tile_pool(name="s64p", bufs=10))
    s32p = ctx.enter_context(tc.tile_pool(name="s32p", bufs=5))
    i32p = ctx.enter_context(tc.tile_pool(name="i32p", bufs=1))
    smi = ctx.enter_context(tc.tile_pool(name="smi", bufs=2))
    omp = ctx.enter_context(tc.tile_pool(name="omp", bufs=1))
    wgt = ctx.enter_context(tc.tile_pool(name="wgt", bufs=1))
    gat = ctx.enter_context(tc.tile_pool(name="gat", bufs=2))
    wbp = ctx.enter_context(tc.tile_pool(name="wbp", bufs=3))
    pp = ctx.enter_context(tc.tile_pool(name="pp", bufs=2))
    qp = ctx.enter_context(tc.tile_pool(name="qp", bufs=2))
    fup = ctx.enter_context(tc.tile_pool(name="fup", bufs=1))
    psd = ctx.enter_context(tc.tile_pool(name="psd", bufs=1, space="PSUM"))
    wbq = ctx.enter_context(tc.tile_pool(name="wbq", bufs=2, space="PSUM"))
    drp = ctx.enter_context(tc.tile_pool(name="drp", bufs=2, space="DRAM"))

    # ---- persistent loads (finit first: first om conv depends on it) ------
    fmas, fsh = [], []
    for h in range(2):
        fm = cst.tile([128, FSZ], F32, tag=f"fmas{h}")
        nc.sync.dma_start(fm[:], dt["finit"][128 * h:128 * (h + 1), :])
        fs = cst.tile([128, FSZ], BF16, tag=f"fsh{h}")
        nc.scalar.copy(fs[:], fm[:])
        fmas.append(fm)
        fsh.append(fs)

    com_t = cst.tile([128, 2 * 9 * 2 * 48], BF16, tag="com")
    nc.sync.dma_start(com_t[:], dt["com_w"])
    com_v = com_t[:].rearrange("p (l t i o) -> p l t i o", l=2, t=9, i=2, o=48)

    byx_t = cst.tile([64, 2 * 480], F32, tag="byx")
    nc.sync.dma_start(byx_t[:], dt["byx"])
    hi0_t = cst.tile([64, 2], F32, tag="hi0")
    nc.sync.dma_start(hi0_t[:], dt["hi0"])
    comb_t = cst.tile([48, 2], F32, tag="comb")
    nc.sync.dma_start(comb_t[:], dt["com_b"])
    dcnb_t = cst.tile([128, 4], F32, tag="dcnb")
    nc.sync.dma_start(dcnb_t[:], dt["dcn_b"])
    resb_t = cst.tile([128, 2], F32, tag="resb")
    nc.sync.dma_start(resb_t[:], dt["res_b"])
    fh_t = cst.tile([128, 2 * RC], BF16, tag="fh")
    nc.sync.dma_start(fh_t[:], dt["fh"])

    # persistent gather-idx tile, both corners: [128, (corner, t, z, cc)]
    rep2 = cst.tile([128, 2 * NT * 64], I16, tag="rep2")
    nc.vector.memset(rep2[:], 0)

    # one-hot row selectors for the PE weight broadcast:
    # oneh[p, zt*128 + o] = (p == zt)
    oneh = cst.tile([32, 32 * 128], BF16, tag="oneh")
    nc.sync.dma_start(oneh[:], dt["oneh"])

    # residual conv weights preloaded (keeps the final conv off the
    # critical path of the last call's weight-slot WAR)
    res_t = cst.tile([128, 9 * 2 * 2 * 128], BF16, tag="res")
    nc.sync.dma_start(res_t[:], dt["res_w"])



    fp_ap = {0: dt["fp1"], 1: dt["fp0"]}

    # ---- per-call ---------------------------------------------------------
    for ci, lvl in enumerate(CALLS):
        k_, st_, pad_, dil_ = CONFIGS[lvl]
        Hin = Win = HIN[lvl]

        # per-call DCN weights
        dcn_t = wgt.tile([128, NT * 2 * 2 * 128], BF16, tag="dcn")
        nc.sync.dma_start(dcn_t[:], dt["dcn_w"][lvl])
        dcn_v = dcn_t[:].rearrange("p (k i o q) -> p k i o q", k=NT, i=2, o=2, q=128)

        # offset conv: om_ps rows 0:48 of a wbq psum slot, [48, (z, 480-of-512)]
        om_full = wbq.tile([128, 1024], F32, tag="wbps", name=f"om_{ci}")
        om_ps = om_full[0:48, :]
        conv3x3(nc, fsh, lambda ti, ih: com_v[:, lvl, ti, ih], om_ps)

        om01 = omp.tile([32, RC], F32, tag="om01")
        omv0 = om_ps[0:32, :].rearrange("p (z c) -> p z c", z=2)[:, :, 0:480]
        nc.scalar.activation(om01[:], omv0, AF.Identity,
                             bias=comb_t[0:32, lvl:lvl + 1])
        m16 = omp.tile([16, RC], F32, tag="m16")
        omv1 = om_ps[32:48, :].rearrange("p (z c) -> p z c", z=2)[:, :, 0:480]
        nc.scalar.activation(m16[:], omv1, AF.Sigmoid,
                             bias=comb_t[32:48, lvl:lvl + 1])

        # shuffle into [64,480] (p = yx*32 + rcb*16 + t) / [32,480]
        pos0 = s64p.tile([64, 480], F32, tag="s64")
        for yx in range(2):
            for rcb in range(2):
                nc.sync.dma_start(
                    pos0[yx * 32 + rcb * 16: yx * 32 + rcb * 16 + 16, :],
                    om01[yx * 16:yx * 16 + 16, rcb * 480:(rcb + 1) * 480])
        m32 = s32p.tile([32, 480], F32, tag="s32")
        for rcb in range(2):
            nc.sync.dma_start(m32[rcb * 16:rcb * 16 + 16, :],
                              m16[:, rcb * 480:(rcb + 1) * 480])

        # ---- small math ----
        cnt = [0]

        def t64():
            cnt[0] += 1
            return s64p.tile([64, 480], F32, tag="s64", name=f"t64_{ci}_{cnt[0]}")

        def t32():
            cnt[0] += 1
            return s32p.tile([32, 480], F32, tag="s32", name=f"t32_{ci}_{cnt[0]}")

        pos = t64()
        nc.vector.tensor_tensor(pos[:], pos0[:],
                                byx_t[:, lvl * 480:(lvl + 1) * 480], A.add)
        sh = t64()
        nc.vector.tensor_scalar(sh[:], pos[:], 1024.0, None, A.add)
        i32t = i32p.tile([64, 480], I32, tag="i32")
        nc.vector.tensor_scalar(i32t[:], pos[:], 1024.0, None, A.add)
        ff = t64()
        nc.vector.tensor_copy(ff[:], i32t[:])
        gt = t64()
        nc.vector.tensor_tensor(gt[:], ff[:], sh[:], A.is_gt)
        fl = t64()
        nc.vector.scalar_tensor_tensor(fl[:], ff[:], -1024.0, gt[:], A.add, A.subtract)
        frac = t64()
        nc.vector.tensor_tensor(frac[:], pos[:], fl[:], A.subtract)
        c0 = t64()
        nc.vector.tensor_scalar(c0[:], fl[:], 0.0, hi0_t[:, lvl:lvl + 1],
                                A.max, A.min)
        t1 = t64()
        nc.vector.tensor_scalar(t1[:], fl[:], 1.0, None, A.add)
        cc1 = t64()
        nc.vector.tensor_scalar(cc1[0:32, :], t1[0:32, :], 0.0, float(Hin - 1),
                                A.max, A.min)
        nc.vector.tensor_copy(cc1[32:64, :], c0[32:64, :])

        # ---- gather indices (emitted early: DVE runs in-order, so the idx
        # DMA chain overlaps the rest of the small math) ----
        # gy written in c' = p'*30+cc column order so DRAM runs are contiguous
        def cin(ap):    # [32,480] rc-ordered input viewed in (p', cc) order
            return ap.rearrange("p (cc pp) -> p pp cc", cc=30, pp=16)

        psx = t32()
        nc.vector.tensor_copy(psx[:], c0[32:64, :])
        gy2 = omp.tile([32, 960], F32, tag="gy2")
        for corner, cy in ((0, c0), (1, cc1)):
            gyv = gy2[:, corner * 480:(corner + 1) * 480].rearrange(
                "p (pp cc) -> p pp cc", pp=16, cc=30)
            nc.vector.scalar_tensor_tensor(gyv, cin(cy[0:32, :]), float(Win),
                                           cin(psx[:]), A.mult, A.add)
        i16t = smi.tile([32, 960], I16, tag="i16")
        nc.scalar.copy(i16t[:], gy2[:])
        # DRAM blocks (corner*2+z) of [p' 16][t 16][cc 30]; dump + wrap with
        # 8x in-DMA replication to all 128 idx partitions (no doubling chain)
        dflat = drp.tile([32, 960], I16, tag="dfl")
        dfv = dflat[:].rearrange("p c -> (p c)")
        r2v = rep2[:].rearrange("p (k t z cc) -> p k t z cc",
                                k=2, t=NT, z=2, cc=32)
        for corner in range(2):
            for z in range(2):
                boff = (corner * 2 + z) * 7680
                dst = dfv[boff:boff + 1].copy()
                dst.ap = bass_rust.VecI64Pair([[30, 16], [480, 16], [1, 30]])
                nc.sync.dma_start(
                    dst, i16t[z * 16:(z + 1) * 16,
                              corner * 480:(corner + 1) * 480])
                src = dfv[boff:boff + 1].copy()
                src.ap = bass_rust.VecI64Pair([[0, 8], [480, 16], [1, 480]])
                nc.sync.dma_start(r2v[:, corner, :, z, 0:30], src)

        V0 = t64()
        nc.vector.tensor_tensor(V0[:], c0[:], fl[:], A.is_equal)
        V1 = t64()
        nc.vector.tensor_tensor(V1[:], cc1[:], t1[:], A.is_equal)
        # x-side ops stay on base-partition 32 (walrus: TT inputs must share
        # base partition); results copied down to base 0 where y-side needs them
        psp = t64()
        nc.vector.tensor_scalar(psp[32:64, :], c0[32:64, :], 1.0, None, A.add)
        F0 = t64()
        nc.vector.tensor_tensor(F0[32:64, :], psp[32:64, :], fl[32:64, :], A.is_equal)
        u = t64()
        nc.vector.tensor_scalar(u[:], frac[:], -1.0, 1.0, A.mult, A.add)
        P0 = t64()
        nc.vector.tensor_tensor(P0[:], u[:], V0[:], A.mult)
        P1 = t64()
        nc.vector.tensor_tensor(P1[:], frac[:], V1[:], A.mult)
        xsA = t64()
        nc.vector.tensor_tensor(xsA[32:64, :], P0[32:64, :], P1[32:64, :], A.add)
        xt = t64()
        nc.vector.tensor_tensor(xt[32:64, :], F0[32:64, :], u[32:64, :], A.mult)
        xt2 = t64()
        nc.vector.tensor_tensor(xt2[32:64, :], V0[32:64, :], frac[32:64, :], A.mult)
        xsB = t64()
        nc.vector.tensor_tensor(xsB[32:64, :], xt[32:64, :], xt2[32:64, :], A.add)
        xs0 = t32()
        nc.vector.tensor_copy(xs0[:], xsA[32:64, :])
        xs1 = t32()
        nc.vector.tensor_copy(xs1[:], xsB[32:64, :])
        A0 = t32()
        nc.vector.tensor_tensor(A0[:], P0[0:32, :], m32[:], A.mult)
        A1 = t32()
        nc.vector.tensor_tensor(A1[:], P1[0:32, :], m32[:], A.mult)

        # wall [32, (slot j, c512)] bf16, j = A(top,pix0) B(top,pix1) C D (bot)
        # 512-col blocks: 480 real + 32 zero pad (sample space padded to 1024/tap)
        wall = smi.tile([32, 4 * 512], BF16, tag="wall")
        nc.vector.tensor_tensor(wall[:, 0 * 512:0 * 512 + 480], A0[:], xs0[:], A.mult)
        nc.vector.tensor_tensor(wall[:, 1 * 512:1 * 512 + 480], A0[:], xs1[:], A.mult)
        nc.vector.tensor_tensor(wall[:, 2 * 512:2 * 512 + 480], A1[:], xs0[:], A.mult)
        nc.vector.tensor_tensor(wall[:, 3 * 512:3 * 512 + 480], A1[:], xs1[:], A.mult)

        # dc accumulator [2][128, 960]
        dcs = [psd.tile([128, 1024], F32, tag=f"dc{oh}", name=f"dc_{ci}_{oh}")
               for oh in range(2)]

        fpv = fp_ap[lvl].copy()
        fpv.ap = bass_rust.VecI64Pair([[C, Hin * Win], [1, 2 * C]])

        RCP = 1024
        for t in range(NT):
            # weight broadcast via PE one-hot matmul + Act drain:
            # wb free (j, z, c512) = wall[z*16+t, j*512+c] on all 128 parts
            wb = wbp.tile([128, 4 * RCP], BF16, tag="wb")
            wb4 = wb[:].rearrange("p (j z c) -> p j z c", j=4, z=2, c=512)
            for z in range(2):
                zt = z * 16 + t
                for jp in range(2):
                    wbps = wbq.tile([128, 1024], F32, tag="wbps",
                                    name=f"wbps_{ci}_{t}_{z}_{jp}")
                    for jh in range(2):
                        nc.tensor.matmul(
                            wbps[:, jh * 512:jh * 512 + 480],
                            oneh[:, zt * 128:(zt + 1) * 128],
                            wall[:, jp * 1024 + jh * 512:
                                 jp * 1024 + jh * 512 + 480],
                            start=True, stop=True)
                    wpv = wbps[:].rearrange("p (j c) -> p j c", j=2)
                    nc.scalar.copy(wb4[:, 2 * jp:2 * jp + 2, z, 0:480],
                                   wpv[:, :, 0:480])
            wbv = wb[:].rearrange("p (j zc) -> p j zc", j=4)

            g2 = gat.tile([128, 2 * 4 * RCP], BF16, tag="g2")
            for corner in range(2):
                gv = g2[:, corner * 4096:(corner + 1) * 4096].rearrange(
                    "p (j i) -> p j i", j=4)
                ioff = corner * NT * 64 + t * 64
                nc.gpsimd.dma_gather(gv, fpv, rep2[:, ioff:ioff + 64],
                                     RCP, RCP, 2 * C, elem_step=C,
                                     transpose=True, single_packet=False)
            g4 = g2[:].rearrange("p (k pix hl z c) -> p k pix hl z c",
                                 k=2, pix=2, hl=2, z=2)
            wbk = wb[:].rearrange("p (k pix z c) -> p k pix z c",
                                  k=2, pix=2, z=2)

            # pad-skip: all DVE ops process only the 480 real cols per z-block
            ss = []
            for hilo in range(2):
                p2 = pp.tile([128, 4 * RCP], BF16, tag="p")
                pv = p2[:].rearrange("p (k pix z c) -> p k pix z c",
                                     k=2, pix=2, z=2)
                nc.vector.tensor_tensor(pv[:, :, :, :, 0:480],
                                        g4[:, :, :, hilo, :, 0:480],
                                        wbk[:, :, :, :, 0:480], A.mult)
                q = qp.tile([128, 2 * RCP], BF16, tag="q")
                qv = q[:].rearrange("p (j z c) -> p j z c", j=2, z=2)
                nc.vector.tensor_tensor(qv[:, :, :, 0:480],
                                        pv[:, 0, :, :, 0:480],
                                        pv[:, 1, :, :, 0:480], A.add)
                ss.append(q)

            # pix-sum absorbed into PE: two matmuls (pix slices of q)
            # accumulate into the same PSUM region
            for oh in range(2):
                for ih in range(2):
                    for nh in range(2):
                        for pix in range(2):
                            nc.tensor.matmul(
                                dcs[oh][:, nh * 512:nh * 512 + 480],
                                dcn_v[:, t, ih, oh],
                                ss[ih][:, pix * 1024 + nh * 512:
                                       pix * 1024 + nh * 512 + 480],
                                start=(t == 0 and ih == 0 and pix == 0),
                                stop=(t == NT - 1 and ih == 1 and pix == 1))

        # f update: f += relu(dc + b); z-major halves so the next call's om
        # conv (nh0 matmuls) can start as soon as rows 1:13 are final
        rels = []
        for h in range(2):
            rel = fup.tile([128, RC], F32, tag=f"rel{h}")
            dcv = dcs[h][:].rearrange("p (z c) -> p z c", z=2)[:, :, 0:480]
            nc.scalar.activation(rel[:], dcv, AF.Relu,
                                 bias=dcnb_t[:, 2 * lvl + h:2 * lvl + h + 1])
            rels.append(rel)
        for zh in range(2):
            for h in range(2):
                fiv = fmas[h][:].rearrange("p (r c) -> p r c", c=FW)[
                    :, 1 + 12 * zh:13 + 12 * zh, 1:41]
                rv = rels[h][:].rearrange("p (r c) -> p r c", c=HOUT)[
                    :, 12 * zh:12 * zh + 12, :]
                nc.vector.tensor_tensor(fiv, fiv, rv, A.add)
                fsv = fsh[h][:].rearrange("p (r c) -> p r c", c=FW)[
                    :, 1 + 12 * zh:13 + 12 * zh, 1:41]
                nc.scalar.copy(fsv, fiv)

    # ---- residual conv + fh ----------------------------------------------
    res_v = res_t[:].rearrange("p (t i o q) -> p t i o q", t=9, i=2, o=2)
    for oh in range(2):
        rps = psd.tile([128, 1024], F32, tag=f"dc{oh}")
        conv3x3(nc, fsh, lambda ti, ih, oh=oh: res_v[:, ti, ih, oh], rps)
        ot = fup.tile([128, RC], F32, tag="ot")
        rpv = rps[:].rearrange("p (z c) -> p z c", z=2)[:, :, 0:480]
        nc.scalar.activation(ot[:], rpv, AF.Identity, bias=resb_t[:, oh:oh + 1])
        nc.vector.tensor_tensor(ot[:], ot[:], fh_t[:, oh * RC:(oh + 1) * RC], A.add)
        nc.sync.dma_start(out_d[128 * oh:128 * (oh + 1), :], ot[:])


def conv3x3(nc, fsh, w_fn, out_ps):
    """3x3 stride-1 conv over the padded f window; out [cout, (z,480-of-512)]
    (PSUM bank limit: a matmul output cannot cross a 512-f32 bank).
    Emitted nh-major, dy-sorted, so the nh0/dy<=0 matmuls only depend on the
    first half of the f update."""
    taps = [(a, b) for a in (-1, 0, 1) for b in (-1, 0, 1)]
    seq = [(ti, dy, dx, ih, nh)
           for nh in range(2)
           for ti, (dy, dx) in enumerate(taps)
           for ih in range(2)]
    for k, (ti, dy, dx, ih, nh) in enumerate(seq):
        rhs = fsh[ih][:].rearrange("p (r c) -> p r c", c=FW)
        nc.tensor.matmul(
            out_ps[:, nh * 512:nh * 512 + 480],
            w_fn(ti, ih),
            rhs[:, 1 + dy + nh * 12:1 + dy + nh * 12 + 12,
                1 + dx:1 + dx + 40],
            start=(ti == 0 and ih == 0), stop=(ti == 8 and ih == 1))


# ===========================================================================
# host side
# ===========================================================================

def prep_core_inputs(inputs, b, half):
    """Per-core input map for image b, row-half `half` (0=top)."""
    g0 = 0 if half == 0 else 16
    f0 = np.asarray(inputs["f0"][b], np.float32)
    f1 = np.asarray(inputs["f1"][b], np.float32)
    f2 = np.asarray(inputs["f2"][b], np.float32)

    def pix_table(f):
        hw = f.shape[1] * f.shape[2]
        t = np.zeros((hw + 1, C), np.float32)
        t[:hw] = f.transpose(1, 2, 0).reshape(hw, C)
        return t.astype(ml_dtypes.bfloat16)

    finit = np.zeros((C, FR, FW), np.float32)
    for r in range(FR):
        gr = g0 - 1 + r
        if 0 <= gr < HOUT:
            finit[:, r, 1:41] = f2[:, gr, :]

    # fh as [128, (oh, rc)]
    fh0 = f2[:, g0:g0 + ROWS, :].reshape(C, RC)
    fh = np.concatenate([fh0[:128], fh0[128:]], axis=1)

    byx = np.zeros((2, 64, 480), np.float32)
    hi0 = np.zeros((2, 64, 1), np.float32)
    for lvl in range(2):
        k_, st_, pad_, dil_ = CONFIGS[lvl]
        Hin = HIN[lvl]
        rc = np.arange(480)
        for rcb in range(2):
            rr = (rcb * 480 + rc) // HOUT
            cc = (rcb * 480 + rc) % HOUT
            for t in range(NT):
                byx[lvl, rcb * 16 + t] = st_ * (g0 + rr) - pad_ + (t // k_) * dil_
                byx[lvl, 32 + rcb * 16 + t] = st_ * cc - pad_ + (t % k_) * dil_
        hi0[lvl, 0:32] = Hin - 1
        hi0[lvl, 32:64] = Hin - 2
    byx = byx.transpose(1, 0, 2).reshape(64, 2 * 480)
    hi0 = hi0.transpose(1, 0, 2).reshape(64, 2)

    perm = list(range(0, 32, 2)) + list(range(1, 32, 2)) + list(range(32, 48))
    com_w = np.zeros((2, 9, 2, 128, 48), np.float32)
    com_b = np.zeros((2, 48, 1), np.float32)
    dcn_w = np.zeros((2, NT, 2, 2, 128, 128), np.float32)
    dcn_b = np.zeros((2, 2, 128, 1), np.float32)
    for lvl in range(2):
        cw = np.asarray(inputs[f"com_w{lvl}"], np.float32)[perm]
        cb = np.asarray(inputs[f"com_b{lvl}"], np.float32)[perm]
        for ty in range(3):
            for tx in range(3):
                for ih in range(2):
                    com_w[lvl, ty * 3 + tx, ih] = \
                        cw[:, ih * 128:(ih + 1) * 128, ty, tx].T
        com_b[lvl, :, 0] = cb
        dw = np.asarray(inputs[f"dcn_w{lvl}"], np.float32)
        for k in range(NT):
            for ih in range(2):
                for oh in range(2):
                    dcn_w[lvl, k, ih, oh] = dw[oh * 128:(oh + 1) * 128,
                                               ih * 128:(ih + 1) * 128,
                                               k // 4, k % 4].T
        db = np.asarray(inputs[f"dcn_b{lvl}"], np.float32)
        dcn_b[lvl, 0, :, 0] = db[:128]
        dcn_b[lvl, 1, :, 0] = db[128:]
    rw = np.asarray(inputs["res_w"], np.float32)
    res_w = np.zeros((9, 2, 2, 128, 128), np.float32)
    for ty in range(3):
        for tx in range(3):
            for ih in range(2):
                for oh in range(2):
                    res_w[ty * 3 + tx, ih, oh] = rw[oh * 128:(oh + 1) * 128,
                                                    ih * 128:(ih + 1) * 128,
                                                    ty, tx].T
    rb = np.asarray(inputs["res_b"], np.float32)
    res_b = np.stack([rb[:128], rb[128:]], axis=1)  # [128, 2]

    # transpose weight stacks to [partition, ...] DRAM layouts
    com_w = com_w.transpose(3, 0, 1, 2, 4).reshape(128, -1)
    com_b = com_b.transpose(1, 0, 2).reshape(48, 2)
    dcn_w = dcn_w.transpose(0, 4, 1, 2, 3, 5).reshape(2, 128, -1)
    dcn_b = dcn_b.transpose(2, 0, 1, 3).reshape(128, 4)
    res_w = res_w.transpose(3, 0, 1, 2, 4).reshape(128, -1)

    oneh = np.zeros((32, 32 * 128), np.float32)
    for zt in range(32):
        oneh[zt, zt * 128:(zt + 1) * 128] = 1.0

    return {
        "oneh": oneh.astype(ml_dtypes.bfloat16),
        "fp0": pix_table(f0),
        "fp1": pix_table(f1),
        "finit": finit.reshape(C, FSZ),
        "fh": fh.astype(ml_dtypes.bfloat16),
        "byx": byx,
        "hi0": hi0,
        "com_w": com_w.astype(ml_dtypes.bfloat16),
        "com_b": np.ascontiguousarray(com_b),
        "dcn_w": np.ascontiguousarray(dcn_w).astype(ml_dtypes.bfloat16),
        "dcn_b": np.ascontiguousarray(dcn_b),
        "res_w": np.ascontiguousarray(res_w).astype(ml_dtypes.bfloat16),
        "res_b": np.ascontiguousarray(res_b).astype(np.float32),
    }


def assemble_output(results):
    out = np.zeros((B, C, HOUT, HOUT), np.float32)
    for b in range(B):
        top = np.asarray(results[2 * b]["out"]).reshape(C, ROWS, HOUT)
        bot = np.asarray(results[2 * b + 1]["out"]).reshape(C, ROWS, HOUT)
        out[b, :, 0:20, :] = top[:, 0:20, :]
        out[b, :, 20:40, :] = bot[:, 4:24, :]
    return out


_NC_CACHE = []


def kernel(**inputs):
    if not _NC_CACHE:
        _NC_CACHE.append(build_program())
    nc = _NC_CACHE[0]
    in_maps = [prep_core_inputs(inputs, b, half)
               for b in range(B) for half in range(2)]
    from concourse.bass_utils import run_bass_kernel_spmd
    r = run_bass_kernel_spmd(nc, in_maps, list(range(8)))
    return assemble_output(r.results)

